# revision 19
# baseline (speedup 1.0000x reference)
"""Trainium2 Bass kernel for the BINN convnet problem.

Computation (per row b of inp, all column indices mod D=128):
    x[b, j]  = (c1[j] * a[b, j+1] - c2[j] * a[b, j-2]) * a[b, j-1]
    out      = x + a @ W_lin.T + b_lin
with c1[j] = w[j,0]*w[j,2], c2[j] = w[j,1]*w[j,2], except j==1 where the
outer factor is w[1,0] instead of w[1,2].

Strategy: pure data parallel across 8 NeuronCores (batch split).  On each
core, per 512-row compute subtile (1024-row DMA tiles, (p q) layout so each
partition line is one contiguous 4 KB DRAM chunk):

  1. PE-transposes A -> A^T per 128-row block (plain fp32 transpose mode);
     ScalarE evacuates PSUM->SBUF, rounding to float32r.
  2. The stencil's linear part g[b,j] = c1[j] a[b,j+1] - c2[j] a[b,j-2] is
     a constant banded matrix G.  One float32r matmul per block with
     lhsT = A^T-block (stationary) and rhs = [G^T | W_lin^T] (moving,
     N=256 -> full PE rate) produces g and mm = a @ W_lin.T both in
     NATURAL layout in PSUM.  No transpose-back is needed.
  3. DVE: x = a[:, j-1] * g with the j-1 roll expressed as shifted
     free-dim access patterns on the natural A tile (main + 1-col wrap),
     then out = x + mm written straight to SBUF.
  4. GpSimd adds the column bias b_lin (broadcast constant) in SBUF.
  5. Store the natural tile.
"""

import os
import sys

import numpy as np

if os.path.isdir("/opt/trn_rl_repo") and "/opt/trn_rl_repo" not in sys.path:
    sys.path.insert(0, "/opt/trn_rl_repo")

import concourse.mybir as mybir
import concourse.tile as tile
from concourse import bacc
from concourse.bass_utils import run_bass_kernel_spmd

D = 128          # feature dim
N_CORES = 8
SUB = 512        # rows per compute subtile
DMA_ROWS = 1024  # rows per DMA tile
F32 = mybir.dt.float32
F32R = mybir.dt.float32r
BIAS_ON_POOL = True


def build_program(nrows: int):
    """Build the single-core Bass program (SPMD across cores)."""
    assert nrows % DMA_ROWS == 0
    ndma = nrows // DMA_ROWS
    nsub = DMA_ROWS // SUB  # compute subtiles per DMA tile (2)
    QB = SUB // D           # 128-row blocks per compute subtile (4)

    nc = bacc.Bacc("TRN2", debug=False, target_bir_lowering=False)

    inp = nc.declare_dram_parameter("inp", [nrows, D], F32, isOutput=False)
    gw = nc.declare_dram_parameter("gw", [D, 2 * D], F32, isOutput=False)
    bbc = nc.declare_dram_parameter("bbc", [D, DMA_ROWS], F32, isOutput=False)
    bmask = nc.declare_dram_parameter("bmask", [1, SUB], F32, isOutput=False)
    ones = nc.declare_dram_parameter("ones", [1, D], F32, isOutput=False)
    ident = nc.declare_dram_parameter("ident", [D, D], F32, isOutput=False)
    out = nc.declare_dram_parameter("out", [nrows, D], F32, isOutput=True)

    with tile.TileContext(nc) as tc:
        with (
            tc.tile_pool(name="const", bufs=1) as const_pool,
            tc.tile_pool(name="a_sb", bufs=3) as a_pool,
            tc.tile_pool(name="at_sb", bufs=3) as at_pool,
            tc.tile_pool(name="xt_sb", bufs=3) as xt_pool,
            tc.tile_pool(name="o_sb", bufs=3) as o_pool,
            tc.tile_pool(name="at_ps", bufs=3, space="PSUM") as atps_pool,
            tc.tile_pool(name="gw_ps", bufs=2, space="PSUM") as gwps_pool,
        ):
            # --- constants, loaded once ---
            gw_sb = const_pool.tile([D, 2 * D], F32)
            bbc_sb = const_pool.tile([D, DMA_ROWS], F32)
            bmask_sb = const_pool.tile([1, SUB], F32)
            ones_sb = const_pool.tile([1, D], F32)
            id_sb = const_pool.tile([D, D], F32)
            nc.sync.dma_start(out=gw_sb[:], in_=gw[:, :])
            nc.sync.dma_start(out=bbc_sb[:], in_=bbc[:, :])
            nc.sync.dma_start(out=bmask_sb[:], in_=bmask[:, :])
            nc.sync.dma_start(out=ones_sb[:], in_=ones[:, :])
            nc.sync.dma_start(out=id_sb[:], in_=ident[:, :])

            # fp32r matmul operands must be produced by an fp32r-rounding
            # instruction (walrus checkMatmultFP32r) — round the constants once.
            gw_rt = const_pool.tile([D, 2 * D], F32R)
            bmask_rt = const_pool.tile([1, SUB], F32R)
            ones_rt = const_pool.tile([1, D], F32R)
            nc.vector.tensor_copy(out=gw_rt[:], in_=gw_sb[:])
            nc.vector.tensor_copy(out=bmask_rt[:], in_=bmask_sb[:])
            nc.vector.tensor_copy(out=ones_rt[:], in_=ones_sb[:])

            for td in range(ndma):
                r0 = td * DMA_ROWS
                # (p q) layout: partition p holds DMA_ROWS/128 consecutive DRAM
                # rows -> each partition line is one contiguous DRAM chunk.
                a_sb = a_pool.tile([D, DMA_ROWS], F32, tag="a")
                src = inp[r0 : r0 + DMA_ROWS, :].rearrange("(p q) d -> p q d", p=D)
                nc.sync.dma_start(
                    out=a_sb[:].rearrange("p (q d) -> p q d", d=D), in_=src
                )
                o_sb = o_pool.tile([D, DMA_ROWS], F32, tag="o")

                for ts in range(nsub):
                    f0 = ts * SUB

                    # --- PE transpose A -> A^T (per 128 block, plain fp32) ---
                    at_ps = atps_pool.tile([D, SUB], F32, tag="atps")
                    for q in range(QB):
                        nc.tensor.matmul(
                            out=at_ps[:, q * D : (q + 1) * D],
                            lhsT=a_sb[:, f0 + q * D : f0 + (q + 1) * D],
                            rhs=id_sb[:],
                            is_transpose=True,
                            start=True,
                            stop=True,
                        )
                    # evacuate A^T to SBUF (ScalarE), rounding to fp32r
                    at_sb = at_pool.tile([D, SUB], F32R, tag="at")
                    nc.scalar.copy(out=at_sb[:], in_=at_ps[:])

                    # --- [g | mm] per block, natural layout, in PSUM ---
                    # gw_ps free layout: [g0|m0|g1|m1|g2|m2|g3|m3], 2 banks
                    gw_ps = gwps_pool.tile([D, 4 * 2 * D], F32, tag="gwps")
                    for q in range(QB):
                        nc.tensor.matmul(
                            out=gw_ps[:, q * 2 * D : (q + 1) * 2 * D],
                            lhsT=at_sb[:, q * D : (q + 1) * D],
                            rhs=gw_rt[:],
                            start=True,
                            stop=BIAS_ON_POOL,
                        )
                    if not BIAS_ON_POOL:
                        # accumulate b_lin onto the mm halves (masked rhs)
                        for h in range(2):
                            nc.tensor.matmul(
                                out=gw_ps[:, h * SUB : (h + 1) * SUB],
                                lhsT=ones_rt[:],
                                rhs=bmask_rt[:],
                                start=False,
                                stop=True,
                            )

                    gw3 = gw_ps[:].rearrange("p (q c) -> p q c", c=2 * D)
                    a3 = a_sb[:, f0 : f0 + SUB].rearrange("p (q d) -> p q d", d=D)
                    o3 = o_sb[:, f0 : f0 + SUB].rearrange("p (q d) -> p q d", d=D)

                    # --- x = a[:, j-1] * g (DVE; shifted free-dim APs) ---
                    xt_sb = xt_pool.tile([D, SUB], F32, tag="xt")
                    x3 = xt_sb[:].rearrange("p (q d) -> p q d", d=D)
                    nc.vector.tensor_mul(
                        out=x3[:, :, 1:D], in0=a3[:, :, 0 : D - 1],
                        in1=gw3[:, :, 1:D],
                    )
                    nc.vector.tensor_mul(
                        out=x3[:, :, 0:1], in0=a3[:, :, D - 1 : D],
                        in1=gw3[:, :, 0:1],
                    )

                    # --- out = x + mm (DVE, straight to SBUF) ---
                    nc.vector.tensor_add(
                        out=o3[:, :, :], in0=xt_sb[:], in1=gw3[:, :, D : 2 * D]
                    )

                if BIAS_ON_POOL:
                    # --- += b_lin broadcast (GpSimd, SBUF only) ---
                    nc.gpsimd.tensor_tensor(
                        out=o_sb[:], in0=o_sb[:], in1=bbc_sb[:],
                        op=mybir.AluOpType.add,
                    )

                # --- store (Scalar HWDGE ring; loads use the SP ring so the
                # two directions don't share one FIFO and one ring's
                # throughput cap) ---
                dst = out[r0 : r0 + DMA_ROWS, :].rearrange("(p q) d -> p q d", p=D)
                nc.scalar.dma_start(
                    out=dst, in_=o_sb[:].rearrange("p (q d) -> p q d", d=D)
                )

    nc.compile()
    return nc


def make_consts(w: np.ndarray, W_lin: np.ndarray, b_lin: np.ndarray):
    """Host-side constant preparation (all tiny)."""
    w = np.asarray(w, np.float64)
    c1 = w[:, 0] * w[:, 2]
    c2 = w[:, 1] * w[:, 2]
    # column 1 uses w[1,0] as the outer factor (faithful to source)
    c1[1] = w[1, 0] * w[1, 0]
    c2[1] = w[1, 1] * w[1, 0]

    j = np.arange(D)
    G = np.zeros((D, D), np.float64)
    G[j, (j + 1) % D] += c1
    G[j, (j - 2) % D] -= c2

    gwm = np.zeros((D, 2 * D), np.float32)
    gwm[:, :D] = G.T           # gw[d, j] = G[j, d]
    gwm[:, D:] = np.asarray(W_lin, np.float64).T  # gw[d, D+j] = W_lin[j, d]

    b32 = np.asarray(b_lin, np.float32)
    bbc = np.ascontiguousarray(np.tile(b32, (D, DMA_ROWS // D)))  # [128, 1024]
    bmask = np.zeros((1, SUB), np.float32)
    bmask[0, D : 2 * D] = b32
    bmask[0, 3 * D : 4 * D] = b32
    ones = np.ones((1, D), np.float32)
    ident = np.eye(D, dtype=np.float32)
    return {"gw": gwm, "bbc": bbc, "bmask": bmask, "ones": ones, "ident": ident}


_PROGRAM_CACHE: dict[int, object] = {}
TRACE = False      # test-only: capture NTFF profile on the next kernel() call
TRACE_DIR = None   # test-only: where to keep NTFF/perfetto artifacts
LAST_RESULT = None  # test-only: BassKernelResults of the last run


def _get_program(nrows: int):
    if nrows not in _PROGRAM_CACHE:
        _PROGRAM_CACHE[nrows] = build_program(nrows)
    return _PROGRAM_CACHE[nrows]


def kernel(**inputs) -> np.ndarray:
    inp = np.ascontiguousarray(np.asarray(inputs["inp"], np.float32))
    w = np.asarray(inputs["w"], np.float32)
    W_lin = np.asarray(inputs["W_lin"], np.float32)
    b_lin = np.asarray(inputs["b_lin"], np.float32)

    B = inp.shape[0]
    assert inp.shape[1] == D and B % N_CORES == 0
    nrows = B // N_CORES

    consts = make_consts(w, W_lin, b_lin)
    shards = inp.reshape(N_CORES, nrows, D)

    nc = _get_program(nrows)
    in_maps = [{"inp": shards[i], **consts} for i in range(N_CORES)]
    res = run_bass_kernel_spmd(
        nc, in_maps, list(range(N_CORES)), trace=TRACE, tmpdir=TRACE_DIR
    )
    global LAST_RESULT
    LAST_RESULT = res
    return np.concatenate([res.results[i]["out"] for i in range(N_CORES)], axis=0)


if __name__ == "__main__":
    # quick smoke test on random data vs numpy
    rng = np.random.default_rng(0)
    B = N_CORES * DMA_ROWS * 2
    inp = rng.standard_normal((B, D)).astype(np.float32)
    w = rng.random((D, 3)).astype(np.float32)
    W_lin = (rng.standard_normal((D, D)) / np.sqrt(D)).astype(np.float32)
    b_lin = (rng.standard_normal(D) * 0.01).astype(np.float32)
    dt = np.ones(1, np.float32)

    actual = kernel(inp=inp, dt=dt, w=w, W_lin=W_lin, b_lin=b_lin)

    a = inp.astype(np.float64)
    c1 = (w[:, 0] * w[:, 2]).astype(np.float64)
    c2 = (w[:, 1] * w[:, 2]).astype(np.float64)
    c1[1] = w[1, 0] * w[1, 0]
    c2[1] = w[1, 1] * w[1, 0]
    ap1 = np.roll(a, -1, 1)
    am2 = np.roll(a, 2, 1)
    am1 = np.roll(a, 1, 1)
    x = (c1 * ap1 - c2 * am2) * am1
    expected = x + a @ W_lin.astype(np.float64).T + b_lin
    err = np.abs(actual - expected).max() / np.abs(expected).max()
    print("scale-relative absmax err:", err)


# revision 22
# speedup vs baseline: 1.0732x; 1.0732x over previous
"""Trainium2 Bass kernel for the BINN convnet problem.

Computation (per row b of inp, all column indices mod D=128):
    x[b, j]  = (c1[j] * a[b, j+1] - c2[j] * a[b, j-2]) * a[b, j-1]
    out      = x + a @ W_lin.T + b_lin
with c1[j] = w[j,0]*w[j,2], c2[j] = w[j,1]*w[j,2], except j==1 where the
outer factor is w[1,0] instead of w[1,2].

Strategy: pure data parallel across 8 NeuronCores (batch split).  On each
core, per 512-row compute subtile (1024-row DMA tiles, (p q) layout so each
partition line is one contiguous 4 KB DRAM chunk):

  1. PE-transposes A -> A^T per 128-row block (plain fp32 transpose mode);
     ScalarE evacuates PSUM->SBUF, rounding to float32r.
  2. The stencil's linear part g[b,j] = c1[j] a[b,j+1] - c2[j] a[b,j-2] is
     a constant banded matrix G.  One float32r matmul per block with
     lhsT = A^T-block (stationary) and rhs = [G^T | W_lin^T] (moving,
     N=256 -> full PE rate) produces g and mm = a @ W_lin.T both in
     NATURAL layout in PSUM.  No transpose-back is needed.
  3. DVE: x = a[:, j-1] * g with the j-1 roll expressed as shifted
     free-dim access patterns on the natural A tile (main + 1-col wrap),
     then out = x + mm written straight to SBUF.
  4. GpSimd adds the column bias b_lin (broadcast constant) in SBUF.
  5. Store the natural tile.
"""

import os
import sys

import numpy as np

if os.path.isdir("/opt/trn_rl_repo") and "/opt/trn_rl_repo" not in sys.path:
    sys.path.insert(0, "/opt/trn_rl_repo")

import concourse.mybir as mybir
import concourse.tile as tile
from concourse import bacc
from concourse.bass_utils import run_bass_kernel_spmd

D = 128          # feature dim
N_CORES = 8
SUB = 512        # rows per compute subtile
DMA_ROWS = 1024  # rows per DMA tile
F32 = mybir.dt.float32
F32R = mybir.dt.float32r
BIAS_ON_POOL = True


def build_program(nrows: int):
    """Build the single-core Bass program (SPMD across cores)."""
    assert nrows % DMA_ROWS == 0
    ndma = nrows // DMA_ROWS
    nsub = DMA_ROWS // SUB  # compute subtiles per DMA tile (2)
    QB = SUB // D           # 128-row blocks per compute subtile (4)

    nc = bacc.Bacc("TRN2", debug=False, target_bir_lowering=False)

    inp = nc.declare_dram_parameter("inp", [nrows, D], F32, isOutput=False)
    gw = nc.declare_dram_parameter("gw", [D, 2 * D], F32, isOutput=False)
    bbc = nc.declare_dram_parameter("bbc", [D, DMA_ROWS], F32, isOutput=False)
    bmask = nc.declare_dram_parameter("bmask", [1, SUB], F32, isOutput=False)
    ones = nc.declare_dram_parameter("ones", [1, D], F32, isOutput=False)
    ident = nc.declare_dram_parameter("ident", [D, D], F32, isOutput=False)
    out = nc.declare_dram_parameter("out", [nrows, D], F32, isOutput=True)

    with tile.TileContext(nc) as tc:
        with (
            tc.tile_pool(name="const", bufs=1) as const_pool,
            tc.tile_pool(name="a_sb", bufs=4) as a_pool,
            tc.tile_pool(name="at_sb", bufs=4) as at_pool,
            tc.tile_pool(name="xt_sb", bufs=4) as xt_pool,
            tc.tile_pool(name="o_sb", bufs=4) as o_pool,
            tc.tile_pool(name="at_ps", bufs=2, space="PSUM") as atps_pool,
            tc.tile_pool(name="gw_ps", bufs=2, space="PSUM") as gwps_pool,
        ):
            # --- constants, loaded once ---
            gw_sb = const_pool.tile([D, 2 * D], F32)
            bbc_sb = const_pool.tile([D, DMA_ROWS], F32)
            bmask_sb = const_pool.tile([1, SUB], F32)
            ones_sb = const_pool.tile([1, D], F32)
            id_sb = const_pool.tile([D, D], F32)
            nc.sync.dma_start(out=gw_sb[:], in_=gw[:, :])
            nc.sync.dma_start(out=bbc_sb[:], in_=bbc[:, :])
            nc.sync.dma_start(out=bmask_sb[:], in_=bmask[:, :])
            nc.sync.dma_start(out=ones_sb[:], in_=ones[:, :])
            nc.sync.dma_start(out=id_sb[:], in_=ident[:, :])

            # fp32r matmul operands must be produced by an fp32r-rounding
            # instruction (walrus checkMatmultFP32r) — round the constants once.
            gw_rt = const_pool.tile([D, 2 * D], F32R)
            bmask_rt = const_pool.tile([1, SUB], F32R)
            ones_rt = const_pool.tile([1, D], F32R)
            nc.vector.tensor_copy(out=gw_rt[:], in_=gw_sb[:])
            nc.vector.tensor_copy(out=bmask_rt[:], in_=bmask_sb[:])
            nc.vector.tensor_copy(out=ones_rt[:], in_=ones_sb[:])

            for td in range(ndma):
                r0 = td * DMA_ROWS
                # (p q) layout: partition p holds DMA_ROWS/128 consecutive DRAM
                # rows -> each partition line is one contiguous DRAM chunk.
                a_sb = a_pool.tile([D, DMA_ROWS], F32, tag="a")
                src = inp[r0 : r0 + DMA_ROWS, :].rearrange("(p q) d -> p q d", p=D)
                nc.sync.dma_start(
                    out=a_sb[:].rearrange("p (q d) -> p q d", d=D), in_=src
                )
                o_sb = o_pool.tile([D, DMA_ROWS], F32, tag="o")

                for ts in range(nsub):
                    f0 = ts * SUB

                    # --- PE transpose A -> A^T (per 128 block, plain fp32) ---
                    at_ps = atps_pool.tile([D, SUB], F32, tag="atps")
                    for q in range(QB):
                        nc.tensor.matmul(
                            out=at_ps[:, q * D : (q + 1) * D],
                            lhsT=a_sb[:, f0 + q * D : f0 + (q + 1) * D],
                            rhs=id_sb[:],
                            is_transpose=True,
                            start=True,
                            stop=True,
                        )
                    # evacuate A^T to SBUF (ScalarE), rounding to fp32r
                    at_sb = at_pool.tile([D, SUB], F32R, tag="at")
                    nc.scalar.copy(out=at_sb[:], in_=at_ps[:])

                    # --- [g | mm] per block, natural layout, in PSUM ---
                    # gw_ps free layout: [g0|m0|g1|m1|g2|m2|g3|m3], 2 banks
                    gw_ps = gwps_pool.tile([D, 4 * 2 * D], F32, tag="gwps")
                    for q in range(QB):
                        nc.tensor.matmul(
                            out=gw_ps[:, q * 2 * D : (q + 1) * 2 * D],
                            lhsT=at_sb[:, q * D : (q + 1) * D],
                            rhs=gw_rt[:],
                            start=True,
                            stop=BIAS_ON_POOL,
                        )
                    if not BIAS_ON_POOL:
                        # accumulate b_lin onto the mm halves (masked rhs)
                        for h in range(2):
                            nc.tensor.matmul(
                                out=gw_ps[:, h * SUB : (h + 1) * SUB],
                                lhsT=ones_rt[:],
                                rhs=bmask_rt[:],
                                start=False,
                                stop=True,
                            )

                    gw3 = gw_ps[:].rearrange("p (q c) -> p q c", c=2 * D)
                    a3 = a_sb[:, f0 : f0 + SUB].rearrange("p (q d) -> p q d", d=D)
                    o3 = o_sb[:, f0 : f0 + SUB].rearrange("p (q d) -> p q d", d=D)

                    # --- x = a[:, j-1] * g (DVE; shifted free-dim APs) ---
                    xt_sb = xt_pool.tile([D, SUB], F32, tag="xt")
                    x3 = xt_sb[:].rearrange("p (q d) -> p q d", d=D)
                    nc.vector.tensor_mul(
                        out=x3[:, :, 1:D], in0=a3[:, :, 0 : D - 1],
                        in1=gw3[:, :, 1:D],
                    )
                    nc.vector.tensor_mul(
                        out=x3[:, :, 0:1], in0=a3[:, :, D - 1 : D],
                        in1=gw3[:, :, 0:1],
                    )

                    # --- out = x + mm (DVE, straight to SBUF) ---
                    nc.vector.tensor_add(
                        out=o3[:, :, :], in0=xt_sb[:], in1=gw3[:, :, D : 2 * D]
                    )

                    if BIAS_ON_POOL:
                        # --- += b_lin broadcast (GpSimd, SBUF only);
                        # per-subtile so it doesn't serialize the DMA tile ---
                        nc.gpsimd.tensor_tensor(
                            out=o_sb[:, f0 : f0 + SUB],
                            in0=o_sb[:, f0 : f0 + SUB],
                            in1=bbc_sb[:, 0:SUB],
                            op=mybir.AluOpType.add,
                        )

                # --- store (Scalar HWDGE ring; loads use the SP ring so the
                # two directions don't share one FIFO and one ring's
                # throughput cap) ---
                dst = out[r0 : r0 + DMA_ROWS, :].rearrange("(p q) d -> p q d", p=D)
                nc.scalar.dma_start(
                    out=dst, in_=o_sb[:].rearrange("p (q d) -> p q d", d=D)
                )

    nc.compile()
    return nc


def make_consts(w: np.ndarray, W_lin: np.ndarray, b_lin: np.ndarray):
    """Host-side constant preparation (all tiny)."""
    w = np.asarray(w, np.float64)
    c1 = w[:, 0] * w[:, 2]
    c2 = w[:, 1] * w[:, 2]
    # column 1 uses w[1,0] as the outer factor (faithful to source)
    c1[1] = w[1, 0] * w[1, 0]
    c2[1] = w[1, 1] * w[1, 0]

    j = np.arange(D)
    G = np.zeros((D, D), np.float64)
    G[j, (j + 1) % D] += c1
    G[j, (j - 2) % D] -= c2

    gwm = np.zeros((D, 2 * D), np.float32)
    gwm[:, :D] = G.T           # gw[d, j] = G[j, d]
    gwm[:, D:] = np.asarray(W_lin, np.float64).T  # gw[d, D+j] = W_lin[j, d]

    b32 = np.asarray(b_lin, np.float32)
    bbc = np.ascontiguousarray(np.tile(b32, (D, DMA_ROWS // D)))  # [128, 1024]
    bmask = np.zeros((1, SUB), np.float32)
    bmask[0, D : 2 * D] = b32
    bmask[0, 3 * D : 4 * D] = b32
    ones = np.ones((1, D), np.float32)
    ident = np.eye(D, dtype=np.float32)
    return {"gw": gwm, "bbc": bbc, "bmask": bmask, "ones": ones, "ident": ident}


_PROGRAM_CACHE: dict[int, object] = {}
TRACE = False      # test-only: capture NTFF profile on the next kernel() call
TRACE_DIR = None   # test-only: where to keep NTFF/perfetto artifacts
LAST_RESULT = None  # test-only: BassKernelResults of the last run


def _get_program(nrows: int):
    if nrows not in _PROGRAM_CACHE:
        _PROGRAM_CACHE[nrows] = build_program(nrows)
    return _PROGRAM_CACHE[nrows]


def kernel(**inputs) -> np.ndarray:
    inp = np.ascontiguousarray(np.asarray(inputs["inp"], np.float32))
    w = np.asarray(inputs["w"], np.float32)
    W_lin = np.asarray(inputs["W_lin"], np.float32)
    b_lin = np.asarray(inputs["b_lin"], np.float32)

    B = inp.shape[0]
    assert inp.shape[1] == D and B % N_CORES == 0
    nrows = B // N_CORES

    consts = make_consts(w, W_lin, b_lin)
    shards = inp.reshape(N_CORES, nrows, D)

    nc = _get_program(nrows)
    in_maps = [{"inp": shards[i], **consts} for i in range(N_CORES)]
    res = run_bass_kernel_spmd(
        nc, in_maps, list(range(N_CORES)), trace=TRACE, tmpdir=TRACE_DIR
    )
    global LAST_RESULT
    LAST_RESULT = res
    return np.concatenate([res.results[i]["out"] for i in range(N_CORES)], axis=0)


if __name__ == "__main__":
    # quick smoke test on random data vs numpy
    rng = np.random.default_rng(0)
    B = N_CORES * DMA_ROWS * 2
    inp = rng.standard_normal((B, D)).astype(np.float32)
    w = rng.random((D, 3)).astype(np.float32)
    W_lin = (rng.standard_normal((D, D)) / np.sqrt(D)).astype(np.float32)
    b_lin = (rng.standard_normal(D) * 0.01).astype(np.float32)
    dt = np.ones(1, np.float32)

    actual = kernel(inp=inp, dt=dt, w=w, W_lin=W_lin, b_lin=b_lin)

    a = inp.astype(np.float64)
    c1 = (w[:, 0] * w[:, 2]).astype(np.float64)
    c2 = (w[:, 1] * w[:, 2]).astype(np.float64)
    c1[1] = w[1, 0] * w[1, 0]
    c2[1] = w[1, 1] * w[1, 0]
    ap1 = np.roll(a, -1, 1)
    am2 = np.roll(a, 2, 1)
    am1 = np.roll(a, 1, 1)
    x = (c1 * ap1 - c2 * am2) * am1
    expected = x + a @ W_lin.astype(np.float64).T + b_lin
    err = np.abs(actual - expected).max() / np.abs(expected).max()
    print("scale-relative absmax err:", err)


# revision 23
# speedup vs baseline: 1.1278x; 1.0509x over previous
"""Trainium2 Bass kernel for the BINN convnet problem.

Computation (per row b of inp, all column indices mod D=128):
    x[b, j]  = (c1[j] * a[b, j+1] - c2[j] * a[b, j-2]) * a[b, j-1]
    out      = x + a @ W_lin.T + b_lin
with c1[j] = w[j,0]*w[j,2], c2[j] = w[j,1]*w[j,2], except j==1 where the
outer factor is w[1,0] instead of w[1,2].

Strategy: pure data parallel across 8 NeuronCores (batch split).  On each
core, per 512-row compute subtile (1024-row DMA tiles, (p q) layout so each
partition line is one contiguous 4 KB DRAM chunk):

  1. PE-transposes A -> A^T per 128-row block (plain fp32 transpose mode);
     ScalarE evacuates PSUM->SBUF, rounding to float32r.
  2. The stencil's linear part g[b,j] = c1[j] a[b,j+1] - c2[j] a[b,j-2] is
     a constant banded matrix G.  One float32r matmul per block with
     lhsT = A^T-block (stationary) and rhs = [G^T | W_lin^T] (moving,
     N=256 -> full PE rate) produces g and mm = a @ W_lin.T both in
     NATURAL layout in PSUM.  No transpose-back is needed.
  3. DVE: x = a[:, j-1] * g with the j-1 roll expressed as shifted
     free-dim access patterns on the natural A tile (main + 1-col wrap),
     then out = x + mm written straight to SBUF.
  4. GpSimd adds the column bias b_lin (broadcast constant) in SBUF.
  5. Store the natural tile.
"""

import os
import sys

import numpy as np

if os.path.isdir("/opt/trn_rl_repo") and "/opt/trn_rl_repo" not in sys.path:
    sys.path.insert(0, "/opt/trn_rl_repo")

import concourse.mybir as mybir
import concourse.tile as tile
from concourse import bacc
from concourse.bass_utils import run_bass_kernel_spmd

D = 128          # feature dim
N_CORES = 8
SUB = 512        # rows per compute subtile
DMA_ROWS = 1024  # rows per DMA tile
F32 = mybir.dt.float32
F32R = mybir.dt.float32r
BIAS_ON_POOL = True


def build_program(nrows: int):
    """Build the single-core Bass program (SPMD across cores)."""
    assert nrows % DMA_ROWS == 0
    ndma = nrows // DMA_ROWS
    nsub = DMA_ROWS // SUB  # compute subtiles per DMA tile (2)
    QB = SUB // D           # 128-row blocks per compute subtile (4)

    nc = bacc.Bacc("TRN2", debug=False, target_bir_lowering=False)

    inp = nc.declare_dram_parameter("inp", [nrows, D], F32, isOutput=False)
    gw = nc.declare_dram_parameter("gw", [D, 2 * D], F32, isOutput=False)
    bbc = nc.declare_dram_parameter("bbc", [D, DMA_ROWS], F32, isOutput=False)
    bmask = nc.declare_dram_parameter("bmask", [1, SUB], F32, isOutput=False)
    ones = nc.declare_dram_parameter("ones", [1, D], F32, isOutput=False)
    ident = nc.declare_dram_parameter("ident", [D, D], F32, isOutput=False)
    out = nc.declare_dram_parameter("out", [nrows, D], F32, isOutput=True)

    with tile.TileContext(nc) as tc:
        with (
            tc.tile_pool(name="const", bufs=1) as const_pool,
            tc.tile_pool(name="a_sb", bufs=4) as a_pool,
            tc.tile_pool(name="at_sb", bufs=4) as at_pool,
            tc.tile_pool(name="xt_sb", bufs=4) as xt_pool,
            tc.tile_pool(name="o_sb", bufs=4) as o_pool,
            tc.tile_pool(name="at_ps", bufs=2, space="PSUM") as atps_pool,
            tc.tile_pool(name="gw_ps", bufs=2, space="PSUM") as gwps_pool,
        ):
            # --- constants, loaded once ---
            gw_sb = const_pool.tile([D, 2 * D], F32)
            bbc_sb = const_pool.tile([D, DMA_ROWS], F32)
            bmask_sb = const_pool.tile([1, SUB], F32)
            ones_sb = const_pool.tile([1, D], F32)
            id_sb = const_pool.tile([D, D], F32)
            nc.sync.dma_start(out=gw_sb[:], in_=gw[:, :])
            nc.sync.dma_start(out=bbc_sb[:], in_=bbc[:, :])
            nc.sync.dma_start(out=bmask_sb[:], in_=bmask[:, :])
            nc.sync.dma_start(out=ones_sb[:], in_=ones[:, :])
            nc.sync.dma_start(out=id_sb[:], in_=ident[:, :])

            # fp32r matmul operands must be produced by an fp32r-rounding
            # instruction (walrus checkMatmultFP32r) — round the constants once.
            gw_rt = const_pool.tile([D, 2 * D], F32R)
            bmask_rt = const_pool.tile([1, SUB], F32R)
            ones_rt = const_pool.tile([1, D], F32R)
            nc.vector.tensor_copy(out=gw_rt[:], in_=gw_sb[:])
            nc.vector.tensor_copy(out=bmask_rt[:], in_=bmask_sb[:])
            nc.vector.tensor_copy(out=ones_rt[:], in_=ones_sb[:])

            # Software pipeline by one subtile: PE's stream per step is
            # [trA(k), GW(k-1)] so PE transposes subtile k while ScalarE
            # evacuates A^T of k-1 — no PE stall on the evac round-trip.
            nsubs = ndma * nsub
            st = {}  # k -> (td, f0, a_sb, o_sb, at_ps, at_sb)

            def emit_front(k):
                td, ts = divmod(k, nsub)
                if ts == 0:
                    r0 = td * DMA_ROWS
                    # (p q) layout: partition p holds DMA_ROWS/128 consecutive
                    # DRAM rows -> one contiguous DRAM chunk per partition.
                    a_sb = a_pool.tile([D, DMA_ROWS], F32, tag="a")
                    src = inp[r0 : r0 + DMA_ROWS, :].rearrange(
                        "(p q) d -> p q d", p=D
                    )
                    nc.sync.dma_start(
                        out=a_sb[:].rearrange("p (q d) -> p q d", d=D), in_=src
                    )
                    o_sb = o_pool.tile([D, DMA_ROWS], F32, tag="o")
                else:
                    _, _, a_sb, o_sb, _, _ = st[k - 1]
                f0 = ts * SUB

                # --- PE transpose A -> A^T (per 128 block, plain fp32) ---
                at_ps = atps_pool.tile([D, SUB], F32, tag="atps")
                for q in range(QB):
                    nc.tensor.matmul(
                        out=at_ps[:, q * D : (q + 1) * D],
                        lhsT=a_sb[:, f0 + q * D : f0 + (q + 1) * D],
                        rhs=id_sb[:],
                        is_transpose=True,
                        start=True,
                        stop=True,
                    )
                st[k] = (td, f0, a_sb, o_sb, at_ps, None)

            def emit_back(k):
                td, f0, a_sb, o_sb, at_ps, _ = st[k]
                ts = k % nsub
                # evacuate A^T to SBUF (ScalarE), rounding to fp32r
                at_sb = at_pool.tile([D, SUB], F32R, tag="at")
                nc.scalar.copy(out=at_sb[:], in_=at_ps[:])

                # --- [g | mm] per block, natural layout, in PSUM ---
                # gw_ps free layout: [g0|m0|g1|m1|g2|m2|g3|m3], 2 banks
                gw_ps = gwps_pool.tile([D, 4 * 2 * D], F32, tag="gwps")
                for q in range(QB):
                    nc.tensor.matmul(
                        out=gw_ps[:, q * 2 * D : (q + 1) * 2 * D],
                        lhsT=at_sb[:, q * D : (q + 1) * D],
                        rhs=gw_rt[:],
                        start=True,
                        stop=BIAS_ON_POOL,
                    )
                if not BIAS_ON_POOL:
                    # accumulate b_lin onto the mm halves (masked rhs)
                    for h in range(2):
                        nc.tensor.matmul(
                            out=gw_ps[:, h * SUB : (h + 1) * SUB],
                            lhsT=ones_rt[:],
                            rhs=bmask_rt[:],
                            start=False,
                            stop=True,
                        )

                gw3 = gw_ps[:].rearrange("p (q c) -> p q c", c=2 * D)
                a3 = a_sb[:, f0 : f0 + SUB].rearrange("p (q d) -> p q d", d=D)
                o3 = o_sb[:, f0 : f0 + SUB].rearrange("p (q d) -> p q d", d=D)

                # --- x = a[:, j-1] * g (DVE; shifted free-dim APs) ---
                xt_sb = xt_pool.tile([D, SUB], F32, tag="xt")
                x3 = xt_sb[:].rearrange("p (q d) -> p q d", d=D)
                nc.vector.tensor_mul(
                    out=x3[:, :, 1:D], in0=a3[:, :, 0 : D - 1], in1=gw3[:, :, 1:D]
                )
                nc.vector.tensor_mul(
                    out=x3[:, :, 0:1], in0=a3[:, :, D - 1 : D], in1=gw3[:, :, 0:1]
                )

                # --- out = x + mm (DVE, straight to SBUF) ---
                nc.vector.tensor_add(
                    out=o3[:, :, :], in0=xt_sb[:], in1=gw3[:, :, D : 2 * D]
                )

                if BIAS_ON_POOL:
                    # --- += b_lin broadcast (GpSimd, SBUF only) ---
                    nc.gpsimd.tensor_tensor(
                        out=o_sb[:, f0 : f0 + SUB],
                        in0=o_sb[:, f0 : f0 + SUB],
                        in1=bbc_sb[:, 0:SUB],
                        op=mybir.AluOpType.add,
                    )

                if ts == nsub - 1:
                    # --- store (Scalar HWDGE ring; loads use the SP ring) ---
                    r0 = td * DMA_ROWS
                    dst = out[r0 : r0 + DMA_ROWS, :].rearrange(
                        "(p q) d -> p q d", p=D
                    )
                    nc.scalar.dma_start(
                        out=dst, in_=o_sb[:].rearrange("p (q d) -> p q d", d=D)
                    )

            emit_front(0)
            for k in range(1, nsubs):
                emit_front(k)
                emit_back(k - 1)
            emit_back(nsubs - 1)

    nc.compile()
    return nc


def make_consts(w: np.ndarray, W_lin: np.ndarray, b_lin: np.ndarray):
    """Host-side constant preparation (all tiny)."""
    w = np.asarray(w, np.float64)
    c1 = w[:, 0] * w[:, 2]
    c2 = w[:, 1] * w[:, 2]
    # column 1 uses w[1,0] as the outer factor (faithful to source)
    c1[1] = w[1, 0] * w[1, 0]
    c2[1] = w[1, 1] * w[1, 0]

    j = np.arange(D)
    G = np.zeros((D, D), np.float64)
    G[j, (j + 1) % D] += c1
    G[j, (j - 2) % D] -= c2

    gwm = np.zeros((D, 2 * D), np.float32)
    gwm[:, :D] = G.T           # gw[d, j] = G[j, d]
    gwm[:, D:] = np.asarray(W_lin, np.float64).T  # gw[d, D+j] = W_lin[j, d]

    b32 = np.asarray(b_lin, np.float32)
    bbc = np.ascontiguousarray(np.tile(b32, (D, DMA_ROWS // D)))  # [128, 1024]
    bmask = np.zeros((1, SUB), np.float32)
    bmask[0, D : 2 * D] = b32
    bmask[0, 3 * D : 4 * D] = b32
    ones = np.ones((1, D), np.float32)
    ident = np.eye(D, dtype=np.float32)
    return {"gw": gwm, "bbc": bbc, "bmask": bmask, "ones": ones, "ident": ident}


_PROGRAM_CACHE: dict[int, object] = {}
TRACE = False      # test-only: capture NTFF profile on the next kernel() call
TRACE_DIR = None   # test-only: where to keep NTFF/perfetto artifacts
LAST_RESULT = None  # test-only: BassKernelResults of the last run


def _get_program(nrows: int):
    if nrows not in _PROGRAM_CACHE:
        _PROGRAM_CACHE[nrows] = build_program(nrows)
    return _PROGRAM_CACHE[nrows]


def kernel(**inputs) -> np.ndarray:
    inp = np.ascontiguousarray(np.asarray(inputs["inp"], np.float32))
    w = np.asarray(inputs["w"], np.float32)
    W_lin = np.asarray(inputs["W_lin"], np.float32)
    b_lin = np.asarray(inputs["b_lin"], np.float32)

    B = inp.shape[0]
    assert inp.shape[1] == D and B % N_CORES == 0
    nrows = B // N_CORES

    consts = make_consts(w, W_lin, b_lin)
    shards = inp.reshape(N_CORES, nrows, D)

    nc = _get_program(nrows)
    in_maps = [{"inp": shards[i], **consts} for i in range(N_CORES)]
    res = run_bass_kernel_spmd(
        nc, in_maps, list(range(N_CORES)), trace=TRACE, tmpdir=TRACE_DIR
    )
    global LAST_RESULT
    LAST_RESULT = res
    return np.concatenate([res.results[i]["out"] for i in range(N_CORES)], axis=0)


if __name__ == "__main__":
    # quick smoke test on random data vs numpy
    rng = np.random.default_rng(0)
    B = N_CORES * DMA_ROWS * 2
    inp = rng.standard_normal((B, D)).astype(np.float32)
    w = rng.random((D, 3)).astype(np.float32)
    W_lin = (rng.standard_normal((D, D)) / np.sqrt(D)).astype(np.float32)
    b_lin = (rng.standard_normal(D) * 0.01).astype(np.float32)
    dt = np.ones(1, np.float32)

    actual = kernel(inp=inp, dt=dt, w=w, W_lin=W_lin, b_lin=b_lin)

    a = inp.astype(np.float64)
    c1 = (w[:, 0] * w[:, 2]).astype(np.float64)
    c2 = (w[:, 1] * w[:, 2]).astype(np.float64)
    c1[1] = w[1, 0] * w[1, 0]
    c2[1] = w[1, 1] * w[1, 0]
    ap1 = np.roll(a, -1, 1)
    am2 = np.roll(a, 2, 1)
    am1 = np.roll(a, 1, 1)
    x = (c1 * ap1 - c2 * am2) * am1
    expected = x + a @ W_lin.astype(np.float64).T + b_lin
    err = np.abs(actual - expected).max() / np.abs(expected).max()
    print("scale-relative absmax err:", err)


# revision 26
# speedup vs baseline: 1.1386x; 1.0095x over previous
"""Trainium2 Bass kernel for the BINN convnet problem.

Computation (per row b of inp, all column indices mod D=128):
    x[b, j]  = (c1[j] * a[b, j+1] - c2[j] * a[b, j-2]) * a[b, j-1]
    out      = x + a @ W_lin.T + b_lin
with c1[j] = w[j,0]*w[j,2], c2[j] = w[j,1]*w[j,2], except j==1 where the
outer factor is w[1,0] instead of w[1,2].

Strategy: pure data parallel across 8 NeuronCores (batch split).  On each
core, per 512-row compute subtile (1024-row DMA tiles, (p q) layout so each
partition line is one contiguous 4 KB DRAM chunk):

  1. PE-transposes A -> A^T per 128-row block (plain fp32 transpose mode);
     ScalarE evacuates PSUM->SBUF, rounding to float32r.
  2. The stencil's linear part g[b,j] = c1[j] a[b,j+1] - c2[j] a[b,j-2] is
     a constant banded matrix G.  One float32r matmul per block with
     lhsT = A^T-block (stationary) and rhs = [G^T | W_lin^T] (moving,
     N=256 -> full PE rate) produces g and mm = a @ W_lin.T both in
     NATURAL layout in PSUM.  No transpose-back is needed.
  3. DVE: x = a[:, j-1] * g with the j-1 roll expressed as shifted
     free-dim access patterns on the natural A tile (main + 1-col wrap),
     then out = x + mm written straight to SBUF.
  4. GpSimd adds the column bias b_lin (broadcast constant) in SBUF.
  5. Store the natural tile.
"""

import os
import sys

import numpy as np

if os.path.isdir("/opt/trn_rl_repo") and "/opt/trn_rl_repo" not in sys.path:
    sys.path.insert(0, "/opt/trn_rl_repo")

import concourse.mybir as mybir
import concourse.tile as tile
from concourse import bacc
from concourse.bass_utils import run_bass_kernel_spmd

D = 128          # feature dim
N_CORES = 8
SUB = 512        # rows per compute subtile
DMA_ROWS = 1024  # rows per DMA tile
F32 = mybir.dt.float32
F32R = mybir.dt.float32r
BIAS_ON_POOL = True


def build_program(nrows: int):
    """Build the single-core Bass program (SPMD across cores)."""
    assert nrows % DMA_ROWS == 0
    ndma = nrows // DMA_ROWS
    nsub = DMA_ROWS // SUB  # compute subtiles per DMA tile (2)
    QB = SUB // D           # 128-row blocks per compute subtile (4)

    nc = bacc.Bacc("TRN2", debug=False, target_bir_lowering=False)

    inp = nc.declare_dram_parameter("inp", [nrows, D], F32, isOutput=False)
    gw = nc.declare_dram_parameter("gw", [D, 2 * D], F32, isOutput=False)
    bbc = nc.declare_dram_parameter("bbc", [D, DMA_ROWS], F32, isOutput=False)
    bmask = nc.declare_dram_parameter("bmask", [1, SUB], F32, isOutput=False)
    ones = nc.declare_dram_parameter("ones", [1, D], F32, isOutput=False)
    ident = nc.declare_dram_parameter("ident", [D, D], F32, isOutput=False)
    out = nc.declare_dram_parameter("out", [nrows, D], F32, isOutput=True)

    with tile.TileContext(nc) as tc:
        with (
            tc.tile_pool(name="const", bufs=1) as const_pool,
            tc.tile_pool(name="a_sb", bufs=4) as a_pool,
            tc.tile_pool(name="at_sb", bufs=4) as at_pool,
            tc.tile_pool(name="xt_sb", bufs=4) as xt_pool,
            tc.tile_pool(name="o_sb", bufs=4) as o_pool,
            tc.tile_pool(name="at_ps", bufs=2, space="PSUM") as atps_pool,
            tc.tile_pool(name="gw_ps", bufs=2, space="PSUM") as gwps_pool,
        ):
            # --- constants, loaded once ---
            gw_sb = const_pool.tile([D, 2 * D], F32)
            bbc_sb = const_pool.tile([D, DMA_ROWS], F32)
            bmask_sb = const_pool.tile([1, SUB], F32)
            ones_sb = const_pool.tile([1, D], F32)
            id_sb = const_pool.tile([D, D], F32)
            nc.sync.dma_start(out=gw_sb[:], in_=gw[:, :])
            nc.sync.dma_start(out=bbc_sb[:], in_=bbc[:, :])
            nc.sync.dma_start(out=bmask_sb[:], in_=bmask[:, :])
            nc.sync.dma_start(out=ones_sb[:], in_=ones[:, :])
            nc.sync.dma_start(out=id_sb[:], in_=ident[:, :])

            # fp32r matmul operands must be produced by an fp32r-rounding
            # instruction (walrus checkMatmultFP32r) — round the constants once.
            gw_rt = const_pool.tile([D, 2 * D], F32R)
            bmask_rt = const_pool.tile([1, SUB], F32R)
            ones_rt = const_pool.tile([1, D], F32R)
            nc.vector.tensor_copy(out=gw_rt[:], in_=gw_sb[:])
            nc.vector.tensor_copy(out=bmask_rt[:], in_=bmask_sb[:])
            nc.vector.tensor_copy(out=ones_rt[:], in_=ones_sb[:])

            # Software pipeline by one subtile: PE's stream per step is
            # [trA(k), GW(k-1)] so PE transposes subtile k while ScalarE
            # evacuates A^T of k-1 — no PE stall on the evac round-trip.
            nsubs = ndma * nsub
            st = {}  # k -> (td, f0, a_sb, o_sb, at_ps, at_sb)

            def emit_front(k):
                td, ts = divmod(k, nsub)
                if ts == 0:
                    r0 = td * DMA_ROWS
                    # (p q) layout: partition p holds DMA_ROWS/128 consecutive
                    # DRAM rows -> one contiguous DRAM chunk per partition.
                    a_sb = a_pool.tile([D, DMA_ROWS], F32, tag="a")
                    src = inp[r0 : r0 + DMA_ROWS, :].rearrange(
                        "(p q) d -> p q d", p=D
                    )
                    nc.sync.dma_start(
                        out=a_sb[:].rearrange("p (q d) -> p q d", d=D), in_=src
                    )
                    o_sb = o_pool.tile([D, DMA_ROWS], F32, tag="o")
                else:
                    _, _, a_sb, o_sb, _, _ = st[k - 1]
                f0 = ts * SUB

                # --- PE transpose A -> A^T (per 128 block, plain fp32) ---
                at_ps = atps_pool.tile([D, SUB], F32, tag="atps")
                for q in range(QB):
                    nc.tensor.matmul(
                        out=at_ps[:, q * D : (q + 1) * D],
                        lhsT=a_sb[:, f0 + q * D : f0 + (q + 1) * D],
                        rhs=id_sb[:],
                        is_transpose=True,
                        start=True,
                        stop=True,
                    )
                st[k] = (td, f0, a_sb, o_sb, at_ps, None)

            def emit_mid(k):
                td, f0, a_sb, o_sb, at_ps, _ = st[k]
                # evacuate A^T to SBUF (ScalarE), rounding to fp32r
                at_sb = at_pool.tile([D, SUB], F32R, tag="at")
                nc.scalar.copy(out=at_sb[:], in_=at_ps[:])

                # --- [g | mm] per block, natural layout, in PSUM ---
                # gw_ps free layout: [g0|m0|g1|m1|g2|m2|g3|m3], 2 banks
                gw_ps = gwps_pool.tile([D, 4 * 2 * D], F32, tag="gwps")
                for q in range(QB):
                    nc.tensor.matmul(
                        out=gw_ps[:, q * 2 * D : (q + 1) * 2 * D],
                        lhsT=at_sb[:, q * D : (q + 1) * D],
                        rhs=gw_rt[:],
                        start=True,
                        stop=BIAS_ON_POOL,
                    )
                if not BIAS_ON_POOL:
                    # accumulate b_lin onto the mm halves (masked rhs)
                    for h in range(2):
                        nc.tensor.matmul(
                            out=gw_ps[:, h * SUB : (h + 1) * SUB],
                            lhsT=ones_rt[:],
                            rhs=bmask_rt[:],
                            start=False,
                            stop=True,
                        )
                st[k] = (td, f0, a_sb, o_sb, at_ps, gw_ps)

            def emit_back(k):
                td, f0, a_sb, o_sb, at_ps, gw_ps = st[k]
                gw3 = gw_ps[:].rearrange("p (q c) -> p q c", c=2 * D)
                a3 = a_sb[:, f0 : f0 + SUB].rearrange("p (q d) -> p q d", d=D)
                o3 = o_sb[:, f0 : f0 + SUB].rearrange("p (q d) -> p q d", d=D)

                # --- x = a[:, j-1] * g (DVE; shifted free-dim APs) ---
                xt_sb = xt_pool.tile([D, SUB], F32, tag="xt")
                x3 = xt_sb[:].rearrange("p (q d) -> p q d", d=D)
                nc.vector.tensor_mul(
                    out=x3[:, :, 1:D], in0=a3[:, :, 0 : D - 1], in1=gw3[:, :, 1:D]
                )
                nc.vector.tensor_mul(
                    out=x3[:, :, 0:1], in0=a3[:, :, D - 1 : D], in1=gw3[:, :, 0:1]
                )

                # --- out = x + mm (DVE, straight to SBUF) ---
                nc.vector.tensor_add(
                    out=o3[:, :, :], in0=xt_sb[:], in1=gw3[:, :, D : 2 * D]
                )

                if BIAS_ON_POOL:
                    # --- += b_lin broadcast (GpSimd, SBUF only) ---
                    nc.gpsimd.tensor_tensor(
                        out=o_sb[:, f0 : f0 + SUB],
                        in0=o_sb[:, f0 : f0 + SUB],
                        in1=bbc_sb[:, 0:SUB],
                        op=mybir.AluOpType.add,
                    )

            def emit_store(k):
                td, _, _, o_sb, _, _ = st[k]
                if k % nsub == nsub - 1:
                    # --- store (Scalar HWDGE ring; loads use the SP ring).
                    # Deferred one extra stage so the store's semaphore wait
                    # (on the GpSimd bias) never stalls ACT's queue ahead of
                    # the next evacuations. ---
                    r0 = td * DMA_ROWS
                    dst = out[r0 : r0 + DMA_ROWS, :].rearrange(
                        "(p q) d -> p q d", p=D
                    )
                    nc.scalar.dma_start(
                        out=dst, in_=o_sb[:].rearrange("p (q d) -> p q d", d=D)
                    )

            # 4-stage pipeline: [trA(k)] [evac+GW(k-1)] [TT+bias(k-2)] [store(k-3)]
            for step in range(nsubs + 3):
                if step < nsubs:
                    emit_front(step)
                if step >= 1 and step - 1 < nsubs:
                    emit_mid(step - 1)
                if step >= 2 and step - 2 < nsubs:
                    emit_back(step - 2)
                if step >= 3 and step - 3 < nsubs:
                    emit_store(step - 3)

    nc.compile()
    return nc


def make_consts(w: np.ndarray, W_lin: np.ndarray, b_lin: np.ndarray):
    """Host-side constant preparation (all tiny)."""
    w = np.asarray(w, np.float64)
    c1 = w[:, 0] * w[:, 2]
    c2 = w[:, 1] * w[:, 2]
    # column 1 uses w[1,0] as the outer factor (faithful to source)
    c1[1] = w[1, 0] * w[1, 0]
    c2[1] = w[1, 1] * w[1, 0]

    j = np.arange(D)
    G = np.zeros((D, D), np.float64)
    G[j, (j + 1) % D] += c1
    G[j, (j - 2) % D] -= c2

    gwm = np.zeros((D, 2 * D), np.float32)
    gwm[:, :D] = G.T           # gw[d, j] = G[j, d]
    gwm[:, D:] = np.asarray(W_lin, np.float64).T  # gw[d, D+j] = W_lin[j, d]

    b32 = np.asarray(b_lin, np.float32)
    bbc = np.ascontiguousarray(np.tile(b32, (D, DMA_ROWS // D)))  # [128, 1024]
    bmask = np.zeros((1, SUB), np.float32)
    bmask[0, D : 2 * D] = b32
    bmask[0, 3 * D : 4 * D] = b32
    ones = np.ones((1, D), np.float32)
    ident = np.eye(D, dtype=np.float32)
    return {"gw": gwm, "bbc": bbc, "bmask": bmask, "ones": ones, "ident": ident}


_PROGRAM_CACHE: dict[int, object] = {}
TRACE = False      # test-only: capture NTFF profile on the next kernel() call
TRACE_DIR = None   # test-only: where to keep NTFF/perfetto artifacts
LAST_RESULT = None  # test-only: BassKernelResults of the last run


def _get_program(nrows: int):
    if nrows not in _PROGRAM_CACHE:
        _PROGRAM_CACHE[nrows] = build_program(nrows)
    return _PROGRAM_CACHE[nrows]


def kernel(**inputs) -> np.ndarray:
    inp = np.ascontiguousarray(np.asarray(inputs["inp"], np.float32))
    w = np.asarray(inputs["w"], np.float32)
    W_lin = np.asarray(inputs["W_lin"], np.float32)
    b_lin = np.asarray(inputs["b_lin"], np.float32)

    B = inp.shape[0]
    assert inp.shape[1] == D and B % N_CORES == 0
    nrows = B // N_CORES

    consts = make_consts(w, W_lin, b_lin)
    shards = inp.reshape(N_CORES, nrows, D)

    nc = _get_program(nrows)
    in_maps = [{"inp": shards[i], **consts} for i in range(N_CORES)]
    res = run_bass_kernel_spmd(
        nc, in_maps, list(range(N_CORES)), trace=TRACE, tmpdir=TRACE_DIR
    )
    global LAST_RESULT
    LAST_RESULT = res
    return np.concatenate([res.results[i]["out"] for i in range(N_CORES)], axis=0)


if __name__ == "__main__":
    # quick smoke test on random data vs numpy
    rng = np.random.default_rng(0)
    B = N_CORES * DMA_ROWS * 2
    inp = rng.standard_normal((B, D)).astype(np.float32)
    w = rng.random((D, 3)).astype(np.float32)
    W_lin = (rng.standard_normal((D, D)) / np.sqrt(D)).astype(np.float32)
    b_lin = (rng.standard_normal(D) * 0.01).astype(np.float32)
    dt = np.ones(1, np.float32)

    actual = kernel(inp=inp, dt=dt, w=w, W_lin=W_lin, b_lin=b_lin)

    a = inp.astype(np.float64)
    c1 = (w[:, 0] * w[:, 2]).astype(np.float64)
    c2 = (w[:, 1] * w[:, 2]).astype(np.float64)
    c1[1] = w[1, 0] * w[1, 0]
    c2[1] = w[1, 1] * w[1, 0]
    ap1 = np.roll(a, -1, 1)
    am2 = np.roll(a, 2, 1)
    am1 = np.roll(a, 1, 1)
    x = (c1 * ap1 - c2 * am2) * am1
    expected = x + a @ W_lin.astype(np.float64).T + b_lin
    err = np.abs(actual - expected).max() / np.abs(expected).max()
    print("scale-relative absmax err:", err)
